# revision 21
# baseline (speedup 1.0000x reference)
"""Trainium2 Bass kernel for a cross-modal transformer block (attention + FFN).

Contract: kernel(**inputs) takes the FULL unsharded inputs (numpy, fp32) and
returns the FULL output [4, 2048, 512] fp32.

Sharding: 8 cores = data-parallel over batch (4) x query-sequence halves (2).
Each core computes K/V projections for its batch's full 2048-token sequence
(cheap duplication) so attention needs no collectives.

v2: fp8 (e4m3) everywhere on the attention side with DoubleRow matmuls for
the K>=256 contractions (QKVO projections, ctx, sumexp); part of the softmax
exp runs on the vector engine via a Schraudolph-style bit trick whose integer
output bits ARE the fp8 exp values; LayerNorm statistics matmuls run in bf16
and rsqrt is computed with a Newton iteration on the vector engine so the
scalar engine only ever loads the Exp and Gelu table sets. The FFN stays in
bf16 for accuracy headroom.
"""

import functools
import sys

import numpy as np

sys.path.insert(0, "/opt/trn_rl_repo")

import ml_dtypes  # noqa: E402

import concourse.bass as bass  # noqa: E402
import concourse.tile as tile  # noqa: E402
from concourse import bacc, mybir  # noqa: E402
from concourse.bass_utils import run_bass_kernel_spmd  # noqa: E402

_orig_tables = bacc.get_activation_tables


def _patched_tables(arch):
    tabs = dict(_orig_tables(arch))
    for name in ("exp_and_others", "exp_and_friends", "natural_log"):
        if name in tabs and "natural_log_exp_and_others" in tabs:
            tabs[name] = set()
    return tabs


bacc.get_activation_tables = _patched_tables

BF16 = mybir.dt.bfloat16
F32 = mybir.dt.float32
FP8 = mybir.dt.float8e4
I32 = mybir.dt.int32
I8 = mybir.dt.int8
AF = mybir.ActivationFunctionType
OP = mybir.AluOpType
DR = mybir.MatmulPerfMode.DoubleRow

B, S, D = 4, 2048, 512
H, DH = 8, 64
FF = 2048
P = 128
C = D // P  # 4 feature chunks
CF = FF // P  # 16 ffn chunks
TQ = S // 2  # 1024 query tokens per core
TK = S  # full key sequence per core
KC = TK // P  # 16 key chunks
KCP = KC // 2  # 8 key chunk pairs (DoubleRow)
NT = 512  # token tile (matmul free dim)
SCALE = 1.0 / np.sqrt(DH)  # 0.125
LN_EPS = 1e-5
NCORES = 8
LN2F = float(np.log(2.0))

# Schraudolph fast-exp constants for fp8e4m3 output bits:
#   bits = round(EXPA * raw_score + EXPB)  ->  ~ 2*exp(raw_score/8)
EXPA = float(8 * np.log2(np.e) * SCALE)
EXPB = 63.62
# key chunks whose exp runs on the vector / gpsimd engine instead of ACT
DVE_KC = (1, 4, 7, 10, 12, 14)
GPS_KC = ()

RSQRT_MAGIC = 0x5F3759DF


def _emit(nc, t, es, tc):
    """Emit the per-core program. t: dict name -> DRAM AP."""
    # ---------------- pools ----------------
    wp = es.enter_context(tc.tile_pool(name="w", bufs=1))
    ap_ = es.enter_context(tc.tile_pool(name="acts", bufs=1))
    ptq = es.enter_context(tc.tile_pool(name="ptq", bufs=2))
    psS = es.enter_context(tc.tile_pool(name="psS", bufs=2, space="PSUM"))
    psX = es.enter_context(tc.tile_pool(name="psX", bufs=3, space="PSUM"))
    psG = es.enter_context(tc.tile_pool(name="psG", bufs=1, space="PSUM"))
    epool = es.enter_context(tc.tile_pool(name="e", bufs=3))
    stage = es.enter_context(tc.tile_pool(name="stage", bufs=2))
    stage1 = es.enter_context(tc.tile_pool(name="stage1", bufs=2))
    sbpool = es.enter_context(tc.tile_pool(name="sb", bufs=2))
    chunk = es.enter_context(tc.tile_pool(name="chunk", bufs=4))
    small = es.enter_context(tc.tile_pool(name="small", bufs=8))
    hpool = es.enter_context(tc.tile_pool(name="h", bufs=1))

    # ---------------- DMA: params + inputs ----------------
    # ordered so kproj can start ASAP: ball/wk/xk first, FFN weights last
    ball = wp.tile([P, 48], F32, name="ball")
    nc.sync.dma_start(ball, t["ball"])
    bq, bk, bo, b2 = (ball[:, 4 * i : 4 * (i + 1)] for i in range(4))
    g1, be1, g2, be2 = (ball[:, 16 + 4 * i : 20 + 4 * i] for i in range(4))
    b1 = ball[:, 32:48]

    def ld(pool, name, shape, dt, split=True):
        w = pool.tile([P] + list(shape), dt, name=name + "_sb")
        if split:
            nc.sync.dma_start(w[0:64], t[name][0:64])
            nc.sync.dma_start(w[64:P], t[name][64:P])
        else:
            nc.sync.dma_start(w, t[name])
        return w

    wk = ld(wp, "wk8", [2, 2, D], FP8, split=False)
    xk = wp.tile([P, 2, 2, TK], FP8, name="xk8_sb")
    for tt in range(4):
        ts_ = slice(tt * NT, (tt + 1) * NT)
        nc.sync.dma_start(xk[:, :, :, ts_], t["xk8"][:, :, :, ts_])
    wq = ld(wp, "wq8", [2, 2, D], FP8, split=False)
    xq8 = ld(wp, "xq8", [2, 2, TQ], FP8, split=False)
    wv = ld(wp, "wv8", [2, 2, D], FP8, split=False)
    xv = ld(wp, "xv8", [2, 2, TK], FP8)
    wo = ld(wp, "wo8", [2, 2, D], FP8, split=False)
    xqr = ld(wp, "xqr", [C, TQ], BF16)
    w1 = ld(wp, "w18", [2, 2, FF], FP8)
    w2 = ld(wp, "w28", [CF // 2, 2, D], FP8)

    onesb = wp.tile([P, 1], BF16)
    nc.vector.memset(onesb, 1.0)
    ln2t = wp.tile([P, 1], F32)
    nc.vector.memset(ln2t, LN2F)
    epst = wp.tile([1, 1], F32)
    nc.vector.memset(epst, LN_EPS)

    # persistent activations
    # ktp: DR-packed, one tile per head pair; head j at partition base 32j,
    # [32j + p, s, key] = K[head j: feat 32s + p, key]
    ktp = [ap_.tile([64, 2, TK], FP8, name=f"ktp_{i}") for i in range(C)]
    va = ap_.tile([P, KCP, 2, H, 72], FP8, name="va")  # V token-major, padded
    nc.vector.memset(va[:, :, :, :, 64:65], 1.0)  # ones column -> sumexp row
    qts = [None, None]
    qtp = [None, None]  # per tq: 2 packed tiles [P, 2, NT]
    ctxs = [None, None]

    out_d = t["out"].rearrange("(c p) q -> p c q", p=P)

    # ---------------- projections ----------------
    def kproj(co, kstage):
        # full-sequence K projection for feature chunk co into a staging tile
        for tt in range(4):
            ts_ = slice(tt * NT, (tt + 1) * NT)
            ps = psX.tile([P, NT], F32, tag="px", name=f"kps_{tt}_{co}")
            for kp in range(2):
                nc.tensor.matmul(
                    ps,
                    wk[:, kp, :, co * P : (co + 1) * P],
                    xk[:, kp, :, ts_],
                    start=(kp == 0),
                    stop=(kp == 1),
                    perf_mode=DR,
                )
            nc.scalar.activation(kstage[:, ts_], ps, AF.Copy)

    def k_repack(co, kstage):
        # staging partitions: head j=0/1 at 64j..64j+63; within a head,
        # feats 0..31 (slot 0) then 32..63 (slot 1).  SBUF->SBUF DMA into
        # the DoubleRow-packed layout ktp[co][32j + p, s, :].
        for j in range(2):
            for s in range(2):
                src = kstage[64 * j + 32 * s : 64 * j + 32 * s + 32, :]
                dst = ktp[co][32 * j : 32 * j + 32, s, :]
                nc.sync.dma_start(dst, src)

    def vproj(tm):
        msl = slice(tm * P, (tm + 1) * P)
        ps = psX.tile([P, NT], F32, tag="px", name=f"vps_{tm}")
        for kp in range(2):
            nc.tensor.matmul(
                ps,
                xv[:, kp, :, msl],
                wv[:, kp, :, :],
                start=(kp == 0),
                stop=(kp == 1),
                perf_mode=DR,
            )
        nc.scalar.activation(
            va[:, tm // 2, tm % 2, :, 0:DH],
            ps.rearrange("p (h d) -> p h d", h=H), AF.Copy,
        )

    def qproj_co(tq, co, qstage):
        ts_ = slice(tq * NT, (tq + 1) * NT)
        ps = psX.tile([P, NT], F32, tag="px", name=f"qps_{tq}_{co}")
        for kp in range(2):
            nc.tensor.matmul(
                ps,
                wq[:, kp, :, co * P : (co + 1) * P],
                xq8[:, kp, :, ts_],
                start=(kp == 0),
                stop=(kp == 1),
                perf_mode=DR,
            )
        nc.vector.tensor_scalar(
            out=qstage, in0=ps, scalar1=bq[:, co : co + 1],
            scalar2=None, op0=OP.add,
        )

    def q_repack(tq, co, qstage):
        for j in range(2):
            for s in range(2):
                src = qstage[64 * j + 32 * s : 64 * j + 32 * s + 32, :]
                dst = qtp[tq][co][32 * j : 32 * j + 32, s, :]
                nc.sync.dma_start(dst, src)

    # ---------------- attention ----------------
    def attn(tq, hp, riders=None):
        ctx = ctxs[tq]
        pcc = [psX.tile([P, NT], F32, tag="px", name=f"pcc_{tq}_{hp}_{j}")
               for j in range(2)]
        e2s = [None] * KCP
        for kcp in range(KCP + 1):
            if kcp < KCP:
                e2t = epool.tile([P, 2, 2, NT], FP8, tag="e", name=f"e_{tq}_{hp}_{kcp}")
                for kcm in range(2):
                    kc = 2 * kcp + kcm
                    ksl = slice(kc * P, (kc + 1) * P)
                    ps2 = psS.tile([P, 2, NT], F32, tag="ps2", name=f"s_{tq}_{hp}_{kc}")
                    for j in range(2):
                        rows = slice(32 * j, 32 * j + 32)
                        nc.tensor.matmul(
                            ps2[:, j, :], ktp[hp][rows, :, ksl],
                            qtp[tq][hp][rows, :, :],
                            start=True, stop=True,
                            perf_mode=DR,
                        )
                    if kc in DVE_KC:
                        e2i = e2t.bitcast(I8)
                        nc.vector.tensor_scalar(
                            out=e2i[:, :, kcm, :], in0=ps2,
                            scalar1=EXPA, scalar2=EXPB, op0=OP.mult, op1=OP.add,
                        )
                    elif kc in GPS_KC:
                        e2i = e2t.bitcast(I8)
                        nc.gpsimd.tensor_scalar(
                            out=e2i[:, :, kcm, :], in0=ps2,
                            scalar1=EXPA, scalar2=EXPB, op0=OP.mult, op1=OP.add,
                        )
                    else:
                        nc.scalar.activation(
                            e2t[:, :, kcm, :], ps2, AF.Exp, bias=ln2t, scale=SCALE,
                        )
                e2s[kcp] = e2t
            if riders:
                riders.pop(0)()
            if kcp >= 1:
                p_ = kcp - 1
                st, sp = (p_ == 0), (p_ == KCP - 1)
                for j in range(2):
                    nc.tensor.matmul(
                        pcc[j][0 : DH + 1, :],
                        va[:, p_, :, 2 * hp + j, 0 : DH + 1],
                        e2s[p_][:, j, :, :],
                        start=st, stop=sp,
                        perf_mode=DR,
                        skip_group_check=True,
                    )
        for j in range(2):
            se = small.tile([1, NT], F32, tag="sm", name=f"se_{tq}_{hp}_{j}")
            nc.vector.tensor_copy(out=se, in_=pcc[j][DH : DH + 1, :])
            rc = small.tile([1, NT], F32, tag="sm", name=f"rc_{tq}_{hp}_{j}")
            nc.vector.reciprocal_approx_fast(out=rc, in_=se)
            db = chunk.tile([DH, NT], F32, tag="db", name=f"db_{tq}_{hp}_{j}")
            nc.gpsimd.partition_broadcast(db, rc)
            nc.vector.tensor_tensor(
                out=ctx[j * DH : (j + 1) * DH, hp, :],
                in0=pcc[j][0:DH, :],
                in1=db,
                op=OP.mult,
            )

    def oproj(tq, resid, co):
        ts_ = slice(tq * NT, (tq + 1) * NT)
        ctx = ctxs[tq]
        ps = psG.tile([P, NT], F32, tag="pg", name=f"ops_{tq}_{co}")
        for kp in range(2):
            nc.tensor.matmul(
                ps,
                wo[:, kp, :, co * P : (co + 1) * P],
                ctx[:, 2 * kp : 2 * kp + 2, :],
                start=(kp == 0),
                stop=(kp == 1),
                perf_mode=DR,
                skip_group_check=True,
            )
        nc.vector.scalar_tensor_tensor(
            out=resid[:, co, :], in0=ps, scalar=bo[:, co : co + 1],
            in1=xqr[:, co, ts_], op0=OP.add, op1=OP.add,
        )

    # ---------------- layernorm (stats in bf16, rsqrt via Newton) --------
    def layernorm(resid, g, be, out_write, tag, out_write_co=None):
        """Normalizes resid IN PLACE (except the final +be which out_write
        directs). resid: [P, C, NT] f32 tile."""
        rb = sbpool.tile([P, C, NT], BF16, tag="rb", name=f"rb_{tag}")
        nc.vector.tensor_copy(out=rb, in_=resid)
        s4 = sbpool.tile([P, C, NT], BF16, tag="rb", name=f"sq_{tag}")
        nc.vector.tensor_mul(s4, rb, rb)
        lnp = psX.tile([P, NT], F32, tag="px", name=f"lnp_{tag}")
        for co in range(C):
            nc.tensor.matmul(lnp[0:1, :], onesb, rb[:, co, :], start=(co == 0),
                             stop=(co == C - 1), skip_group_check=True)
        for co in range(C):
            nc.tensor.matmul(lnp[64:65, :], onesb, s4[:, co, :], start=(co == 0),
                             stop=(co == C - 1), tile_position=(0, 64),
                             skip_group_check=True)
        mean = small.tile([1, NT], F32, tag="sm", name=f"mean_{tag}")
        nc.vector.tensor_scalar_mul(mean, lnp[0:1, :], 1.0 / D)
        msq = small.tile([1, NT], F32, tag="sm", name=f"msq_{tag}")
        nc.vector.tensor_scalar_mul(msq, lnp[64:65, :], 1.0 / D)
        m2 = small.tile([1, NT], F32, tag="sm", name=f"m2_{tag}")
        nc.vector.tensor_mul(m2, mean, mean)
        var = small.tile([1, NT], F32, tag="sm", name=f"var_{tag}")
        nc.vector.tensor_tensor(out=var, in0=msq, in1=m2, op=OP.subtract)
        # rstd = exp(-0.5 * ln(var + eps)) -- stays in the Exp/Ln ACT table set
        lnv = small.tile([1, NT], F32, tag="sm", name=f"lnv_{tag}")
        nc.scalar.activation(lnv, var, AF.Ln, bias=epst)
        rstd = small.tile([1, NT], F32, tag="sm", name=f"rstd_{tag}")
        nc.scalar.activation(rstd, lnv, AF.Exp, scale=-0.5)
        meanb = chunk.tile([P, NT], F32, tag="bc", name=f"meanb_{tag}")
        nc.gpsimd.partition_broadcast(meanb, mean)
        rstdb = chunk.tile([P, NT], F32, tag="bc", name=f"rstdb_{tag}")
        nc.gpsimd.partition_broadcast(rstdb, rstd)
        if out_write_co is not None:
            for co in range(C):
                nc.vector.tensor_tensor(
                    out=resid[:, co, :], in0=resid[:, co, :], in1=meanb,
                    op=OP.subtract,
                )
                nc.vector.scalar_tensor_tensor(
                    out=resid[:, co, :], in0=resid[:, co, :],
                    scalar=g[:, co : co + 1], in1=rstdb, op0=OP.mult, op1=OP.mult,
                )
                out_write_co(co, resid[:, co, :], be[:, co : co + 1])
            return
        nc.vector.tensor_tensor(
            out=resid, in0=resid,
            in1=meanb[:, None, :].to_broadcast((P, C, NT)), op=OP.subtract,
        )
        nc.vector.tensor_tensor(
            out=resid, in0=resid,
            in1=rstdb[:, None, :].to_broadcast((P, C, NT)), op=OP.mult,
        )
        nc.vector.tensor_tensor(
            out=resid, in0=resid,
            in1=g[:, :, None].to_broadcast((P, C, NT)), op=OP.mult,
        )
        out_write(resid, be)

    # ---------------- FFN ----------------
    def ffn1(tq, ln1b, hb):
        for fp in range(CF // 2):
            ps = psS.tile([P, 2, NT], F32, tag="ps2", name=f"fps_{tq}_{fp}")
            for f2 in range(2):
                fo = 2 * fp + f2
                for kp in range(2):
                    nc.tensor.matmul(
                        ps[:, f2, :],
                        w1[:, kp, :, fo * P : (fo + 1) * P],
                        ln1b[:, 2 * kp : 2 * kp + 2, :],
                        start=(kp == 0),
                        stop=(kp == 1),
                        perf_mode=DR,
                    )
            for f2 in range(2):
                nc.scalar.activation(
                    hb[:, 2 * fp + f2, :], ps[:, f2, :], AF.Gelu,
                    bias=b1[:, 2 * fp + f2 : 2 * fp + f2 + 1],
                )

    def ffn2(tq, ln1f, hb):
        resid2 = stage1.tile([P, C, NT], F32, tag="resid2", name=f"resid2_{tq}")
        for co in range(C):
            ps = psX.tile([P, NT], F32, tag="px", name=f"gps_{tq}_{co}")
            for kp in range(CF // 2):
                nc.tensor.matmul(
                    ps,
                    w2[:, kp, :, co * P : (co + 1) * P],
                    hb[:, 2 * kp : 2 * kp + 2, :],
                    start=(kp == 0),
                    stop=(kp == CF // 2 - 1),
                    perf_mode=DR,
                    skip_group_check=True,
                )
            nc.vector.scalar_tensor_tensor(
                out=resid2[:, co, :], in0=ps, scalar=b2[:, co : co + 1],
                in1=ln1f[:, co, :], op0=OP.add, op1=OP.add,
            )
        return resid2

    def ln2(tq, resid2):
        ts_ = slice(tq * NT, (tq + 1) * NT)

        def write_out_co(co, v, bec, ts_=ts_):
            nc.vector.tensor_scalar(
                out=v, in0=v, scalar1=bec, scalar2=None, op0=OP.add
            )
            nc.sync.dma_start(out_d[:, co, ts_], v)

        layernorm(resid2, g2, be2, None, f"l2_{tq}", out_write_co=write_out_co)

    # ================= schedule =================
    for co in range(C):
        kstage = stage.tile([P, TK], FP8, tag="kstage", name=f"kst_{co}")
        kproj(co, kstage)
        k_repack(co, kstage)
    qtp[0] = [ptq.tile([64, 2, NT], FP8, tag=f"qtp{i}", name=f"qtp_0_{i}")
              for i in range(C)]
    for co in range(C):
        qstage = stage.tile([P, NT], FP8, tag="qstage", name=f"qst_0_{co}")
        qproj_co(0, co, qstage)
        q_repack(0, co, qstage)
    qtp[1] = [ptq.tile([64, 2, NT], FP8, tag=f"qtp{i}", name=f"qtp_1_{i}")
              for i in range(C)]
    for co in range(C):
        qstage = stage.tile([P, NT], FP8, tag="qstage", name=f"qst_1_{co}")
        qproj_co(1, co, qstage)
        q_repack(1, co, qstage)
    for tm in range(KC):
        vproj(tm)

    # ---- attention tq0; Qproj(tq1) rides ----
    ctxs[0] = ptq.tile([P, C, NT], FP8, tag="ctx", name="ctx_0")
    nop = lambda: None
    for hp in range(H // 2):
        attn(0, hp)

    # ---- attention tq1; Oproj(0) + LN1(0) ride ----
    ctxs[1] = ptq.tile([P, C, NT], FP8, tag="ctx", name="ctx_1")
    resid0 = stage.tile([P, C, NT], F32, tag="resid", name="resid_0")
    ln1f0 = stage.tile([P, C, NT], F32, tag="ln1f", name="ln1f_0")
    ln1b0 = ptq.tile([P, C, NT], FP8, tag="ln1b", name="ln1b_0")

    def write_ln1_0(tt, be):
        nc.vector.tensor_tensor(
            out=ln1f0, in0=tt,
            in1=be[:, :, None].to_broadcast((P, C, NT)), op=OP.add,
        )
        nc.vector.tensor_copy(out=ln1b0, in_=ln1f0)

    for hp in range(H // 2):
        riders = [nop] * KCP
        if hp == 0:
            riders[2] = lambda: oproj(0, resid0, 0)
            riders[5] = lambda: oproj(0, resid0, 1)
        elif hp == 1:
            riders[2] = lambda: oproj(0, resid0, 2)
            riders[5] = lambda: oproj(0, resid0, 3)
        elif hp == 2:
            riders[3] = lambda: layernorm(resid0, g1, be1, write_ln1_0, "l1_0")
        attn(1, hp, riders)

    # ---- tails: FFN + LN chains, interleaved so each serial LN chain
    # hides under the next block's matmuls ----
    resid1 = stage.tile([P, C, NT], F32, tag="resid", name="resid_1")
    for co in range(C):
        oproj(1, resid1, co)

    ln1f1 = stage.tile([P, C, NT], F32, tag="ln1f", name="ln1f_1")
    ln1b1 = ptq.tile([P, C, NT], FP8, tag="ln1b", name="ln1b_1")

    def write_ln1_1(tt, be):
        nc.vector.tensor_tensor(
            out=ln1f1, in0=tt,
            in1=be[:, :, None].to_broadcast((P, C, NT)), op=OP.add,
        )
        nc.vector.tensor_copy(out=ln1b1, in_=ln1f1)

    layernorm(resid1, g1, be1, write_ln1_1, "l1_1")
    hb0 = hpool.tile([P, CF, NT], FP8, tag="h", name="h_0")
    ffn1(0, ln1b0, hb0)
    r2_0 = ffn2(0, ln1f0, hb0)
    ln2(0, r2_0)
    hb1 = hpool.tile([P, CF, NT], FP8, tag="h", name="h_1")
    ffn1(1, ln1b1, hb1)
    r2_1 = ffn2(1, ln1f1, hb1)
    ln2(1, r2_1)


@functools.lru_cache(maxsize=1)
def build():
    from contextlib import ExitStack

    nc = bacc.Bacc("TRN2", target_bir_lowering=False, debug=False, num_devices=NCORES)
    t = {}

    def din(name, shape, dt):
        t[name] = nc.dram_tensor(name, list(shape), dt, kind="ExternalInput").ap()

    din("xq8", (P, 2, 2, TQ), FP8)
    din("xqr", (P, C, TQ), BF16)
    din("xk8", (P, 2, 2, TK), FP8)
    din("xv8", (P, 2, 2, TK), FP8)
    for w in ("wq8", "wk8", "wv8", "wo8"):
        din(w, (P, 2, 2, D), FP8)
    din("w18", (P, 2, 2, FF), FP8)
    din("w28", (P, CF // 2, 2, D), FP8)
    din("ball", (P, 48), F32)
    t["out"] = nc.dram_tensor("out", [D, TQ], F32, kind="ExternalOutput").ap()

    with tile.TileContext(nc) as tc:
        with ExitStack() as es:
            _emit(nc, t, es, tc)
    nc.compile()
    return nc


def make_in_maps(query, key, value, Wq, bq, Wk, bk, Wv, bv, Wo, bo,
                 g1, be1, g2, be2, W1, b1, W2, b2):
    bf = ml_dtypes.bfloat16
    f8 = ml_dtypes.float8_e4m3

    def pmaj(w, dt=bf):
        # [K, N] -> partition-major [128, K//128, N]
        w = np.asarray(w)
        k, n = w.shape
        return np.ascontiguousarray(
            w.reshape(k // P, P, n).transpose(1, 0, 2).astype(dt)
        )

    def pmaj_dr(w):
        # [K, N] -> DoubleRow layout [128, K//256, 2, N] fp8
        w = np.asarray(w, np.float32)
        k, n = w.shape
        return np.ascontiguousarray(
            np.clip(w, -240, 240).reshape(k // 256, 2, P, n)
            .transpose(2, 0, 1, 3).astype(f8)
        )

    bo2 = np.asarray(bo, np.float32) + np.asarray(bv, np.float32) @ np.asarray(Wo, np.float32)
    cols = [np.asarray(v, np.float32).reshape(-1, P).T
            for v in (bq, bk, bo2, b2, g1, be1, g2, be2, b1)]
    ball = np.ascontiguousarray(np.concatenate(cols, axis=1))  # [128, 48]
    shared = {
        "ball": ball,
        "wq8": pmaj_dr(Wq), "wk8": pmaj_dr(Wk), "wv8": pmaj_dr(Wv),
        "wo8": pmaj_dr(Wo),
        "w18": pmaj_dr(W1), "w28": pmaj_dr(W2),
    }
    in_maps = []
    for core in range(NCORES):
        b, half = divmod(core, 2)
        qsl = slice(half * TQ, (half + 1) * TQ)
        xq_t = np.asarray(query[b, qsl], np.float32).T  # [D, TQ]
        in_maps.append({
            "xq8": pmaj_dr(xq_t), "xqr": pmaj(xq_t),
            "xk8": pmaj_dr(np.asarray(key[b], np.float32).T),
            "xv8": pmaj_dr(np.asarray(value[b], np.float32).T), **shared,
        })
    return in_maps


def kernel(**inputs):
    nc = build()
    in_maps = make_in_maps(**inputs)
    res = run_bass_kernel_spmd(nc, in_maps, list(range(NCORES)))
    out = np.empty((B, S, D), np.float32)
    for core in range(NCORES):
        b, half = divmod(core, 2)
        out[b, half * TQ : (half + 1) * TQ] = res.results[core]["out"].T
    return out


if __name__ == "__main__":
    import reference

    inputs = {k: np.asarray(v) for k, v in reference.setup_inputs().items()}
    got = kernel(**inputs)
    exp = np.asarray(reference.reference(**inputs))
    err = np.abs(got - exp).max() / np.abs(exp).max()
    print("rel err:", err)



# revision 28
# speedup vs baseline: 1.0844x; 1.0844x over previous
"""Trainium2 Bass kernel for a cross-modal transformer block (attention + FFN).

Contract: kernel(**inputs) takes the FULL unsharded inputs (numpy, fp32) and
returns the FULL output [4, 2048, 512] fp32.

Sharding: 8 cores = data-parallel over batch (4) x query-sequence halves (2).
Each core computes K/V projections for its batch's full 2048-token sequence
(cheap duplication) so attention needs no collectives.

v2: fp8 (e4m3) everywhere on the attention side with DoubleRow matmuls for
the K>=256 contractions (QKVO projections, ctx, sumexp); part of the softmax
exp runs on the vector engine via a Schraudolph-style bit trick whose integer
output bits ARE the fp8 exp values; LayerNorm statistics matmuls run in bf16
and rsqrt is computed with a Newton iteration on the vector engine so the
scalar engine only ever loads the Exp and Gelu table sets. The FFN stays in
bf16 for accuracy headroom.
"""

import functools
import sys

import numpy as np

sys.path.insert(0, "/opt/trn_rl_repo")

import ml_dtypes  # noqa: E402

import concourse.bass as bass  # noqa: E402
import concourse.tile as tile  # noqa: E402
from concourse import bacc, mybir  # noqa: E402
from concourse.bass_utils import run_bass_kernel_spmd  # noqa: E402

_orig_tables = bacc.get_activation_tables


def _patched_tables(arch):
    tabs = dict(_orig_tables(arch))
    for name in ("exp_and_others", "exp_and_friends", "natural_log"):
        if name in tabs and "natural_log_exp_and_others" in tabs:
            tabs[name] = set()
    return tabs


bacc.get_activation_tables = _patched_tables

BF16 = mybir.dt.bfloat16
F32 = mybir.dt.float32
FP8 = mybir.dt.float8e4
I32 = mybir.dt.int32
I8 = mybir.dt.int8
AF = mybir.ActivationFunctionType
OP = mybir.AluOpType
DR = mybir.MatmulPerfMode.DoubleRow

B, S, D = 4, 2048, 512
H, DH = 8, 64
FF = 2048
P = 128
C = D // P  # 4 feature chunks
CF = FF // P  # 16 ffn chunks
TQ = S // 2  # 1024 query tokens per core
TK = S  # full key sequence per core
KC = TK // P  # 16 key chunks
KCP = KC // 2  # 8 key chunk pairs (DoubleRow)
NT = 512  # token tile (matmul free dim)
SCALE = 1.0 / np.sqrt(DH)  # 0.125
LN_EPS = 1e-5
NCORES = 8
LN2F = float(np.log(2.0))

# Schraudolph fast-exp constants for fp8e4m3 output bits:
#   bits = round(EXPA * raw_score + EXPB)  ->  ~ 2*exp(raw_score/8)
EXPA = float(8 * np.log2(np.e) * SCALE)
EXPB = 63.62
# key chunks whose exp runs on the vector / gpsimd engine instead of ACT
DVE_KC = (1, 4, 7, 10, 12, 14)
GPS_KC = ()

RSQRT_MAGIC = 0x5F3759DF


def _emit(nc, t, es, tc):
    """Emit the per-core program. t: dict name -> DRAM AP."""
    # ---------------- pools ----------------
    wp = es.enter_context(tc.tile_pool(name="w", bufs=1))
    ap_ = es.enter_context(tc.tile_pool(name="acts", bufs=1))
    ptq = es.enter_context(tc.tile_pool(name="ptq", bufs=2))
    psS = es.enter_context(tc.tile_pool(name="psS", bufs=2, space="PSUM"))
    psX = es.enter_context(tc.tile_pool(name="psX", bufs=3, space="PSUM"))
    psG = es.enter_context(tc.tile_pool(name="psG", bufs=1, space="PSUM"))
    epool = es.enter_context(tc.tile_pool(name="e", bufs=3))
    stage = es.enter_context(tc.tile_pool(name="stage", bufs=2))
    stage1 = es.enter_context(tc.tile_pool(name="stage1", bufs=2))
    sbpool = es.enter_context(tc.tile_pool(name="sb", bufs=2))
    chunk = es.enter_context(tc.tile_pool(name="chunk", bufs=4))
    small = es.enter_context(tc.tile_pool(name="small", bufs=8))
    hpool = es.enter_context(tc.tile_pool(name="h", bufs=1))

    # ---------------- DMA: params + inputs ----------------
    # ordered so kproj can start ASAP: ball/wk/xk first, FFN weights last
    ball = wp.tile([P, 56], F32, name="ball")
    nc.sync.dma_start(ball, t["ball"])
    bq, bk, bo, b2 = (ball[:, 4 * i : 4 * (i + 1)] for i in range(4))
    g1, be1, g2, be2 = (ball[:, 16 + 4 * i : 20 + 4 * i] for i in range(4))
    b1 = ball[:, 32:48]
    bqp = ball[:, 48:56]  # bq in DR-packed order, [64 used, 2*co + s]

    def ld(pool, name, shape, dt, split=True):
        w = pool.tile([P] + list(shape), dt, name=name + "_sb")
        if split:
            nc.sync.dma_start(w[0:64], t[name][0:64])
            nc.sync.dma_start(w[64:P], t[name][64:P])
        else:
            nc.sync.dma_start(w, t[name])
        return w

    wk = ld(wp, "wk8", [2, 2, D], FP8, split=False)
    xk = wp.tile([P, 2, 2, TK], FP8, name="xk8_sb")
    for tt in range(4):
        ts_ = slice(tt * NT, (tt + 1) * NT)
        nc.sync.dma_start(xk[:, :, :, ts_], t["xk8"][:, :, :, ts_])
    wq = ld(wp, "wq8", [2, 2, D], FP8, split=False)
    xq8 = ld(wp, "xq8", [2, 2, TQ], FP8, split=False)
    wv = ld(wp, "wv8", [2, 2, D], FP8, split=False)
    xv = ld(wp, "xv8", [2, 2, TK], FP8)
    wo = ld(wp, "wo8", [2, 2, D], FP8, split=False)
    xqr = ld(wp, "xqr", [C, TQ], BF16)
    w1 = ld(wp, "w18", [2, 2, FF], FP8)
    w2 = ld(wp, "w28", [CF // 2, 2, D], FP8)

    onesb = wp.tile([P, 1], BF16)
    nc.vector.memset(onesb, 1.0)
    ln2t = wp.tile([P, 1], F32)
    nc.vector.memset(ln2t, LN2F)
    epst = wp.tile([1, 1], F32)
    nc.vector.memset(epst, LN_EPS)

    # persistent activations
    # ktp: DR-packed, one tile per head pair; head j at partition base 32j,
    # [32j + p, s, key] = K[head j: feat 32s + p, key]
    ktp = [ap_.tile([64, 2, TK], FP8, name=f"ktp_{i}") for i in range(C)]
    va = ap_.tile([P, KCP, 2, H, 72], FP8, name="va")  # V token-major, padded
    nc.vector.memset(va[:, :, :, :, 64:65], 1.0)  # ones column -> sumexp row
    qts = [None, None]
    qtp = [None, None]  # per tq: 2 packed tiles [P, 2, NT]
    ctxs = [None, None]

    out_d = t["out"].rearrange("(c p) q -> p c q", p=P)

    # ---------------- projections ----------------
    # Wk/Wq columns are host-reordered per 128-chunk so the first 64 columns
    # are [head0 f0:32 | head1 f0:32] (DR slot 0) and the last 64 are
    # [head0 f32:64 | head1 f32:64] (slot 1).  Two M=64 matmuls per chunk
    # then land the packed layout with ONE partition-aligned copy.
    def kproj(tt, co):
        ts_ = slice(tt * NT, (tt + 1) * NT)
        ps = psS.tile([P, 2, NT], F32, tag="ps2", name=f"kps_{tt}_{co}")
        for s in range(2):
            for kp in range(2):
                nc.tensor.matmul(
                    ps[0:64, s, :],
                    wk[:, kp, :, co * P + 64 * s : co * P + 64 * s + 64],
                    xk[:, kp, :, ts_],
                    start=(kp == 0),
                    stop=(kp == 1),
                    perf_mode=DR,
                )
        nc.scalar.activation(ktp[co][:, :, ts_], ps[0:64], AF.Copy)

    def vproj(tm):
        msl = slice(tm * P, (tm + 1) * P)
        ps = psX.tile([P, NT], F32, tag="px", name=f"vps_{tm}")
        for kp in range(2):
            nc.tensor.matmul(
                ps,
                xv[:, kp, :, msl],
                wv[:, kp, :, :],
                start=(kp == 0),
                stop=(kp == 1),
                perf_mode=DR,
            )
        nc.scalar.activation(
            va[:, tm // 2, tm % 2, :, 0:DH],
            ps.rearrange("p (h d) -> p h d", h=H), AF.Copy,
        )

    def qproj_co(tq, co):
        ts_ = slice(tq * NT, (tq + 1) * NT)
        ps = psS.tile([P, 2, NT], F32, tag="ps2", name=f"qps_{tq}_{co}")
        for s in range(2):
            for kp in range(2):
                nc.tensor.matmul(
                    ps[0:64, s, :],
                    wq[:, kp, :, co * P + 64 * s : co * P + 64 * s + 64],
                    xq8[:, kp, :, ts_],
                    start=(kp == 0),
                    stop=(kp == 1),
                    perf_mode=DR,
                )
        for s in range(2):
            nc.vector.tensor_scalar(
                out=qtp[tq][co][:, s, :], in0=ps[0:64, s, :],
                scalar1=bqp[0:64, 2 * co + s : 2 * co + s + 1],
                scalar2=None, op0=OP.add,
            )

    # ---------------- attention ----------------
    def attn(tq, hp, riders=None):
        ctx = ctxs[tq]
        pcc = [psX.tile([P, NT], F32, tag="px", name=f"pcc_{tq}_{hp}_{j}")
               for j in range(2)]
        e2s = [None] * KCP
        for kcp in range(KCP + 1):
            if kcp < KCP:
                e2t = epool.tile([P, 2, 2, NT], FP8, tag="e", name=f"e_{tq}_{hp}_{kcp}")
                for kcm in range(2):
                    kc = 2 * kcp + kcm
                    ksl = slice(kc * P, (kc + 1) * P)
                    ps2 = psS.tile([P, 2, NT], F32, tag="ps2", name=f"s_{tq}_{hp}_{kc}")
                    for j in range(2):
                        rows = slice(32 * j, 32 * j + 32)
                        nc.tensor.matmul(
                            ps2[:, j, :], ktp[hp][rows, :, ksl],
                            qtp[tq][hp][rows, :, :],
                            start=True, stop=True,
                            perf_mode=DR,
                        )
                    if kc in DVE_KC:
                        e2i = e2t.bitcast(I8)
                        nc.vector.tensor_scalar(
                            out=e2i[:, :, kcm, :], in0=ps2,
                            scalar1=EXPA, scalar2=EXPB, op0=OP.mult, op1=OP.add,
                        )
                    elif kc in GPS_KC:
                        e2i = e2t.bitcast(I8)
                        nc.gpsimd.tensor_scalar(
                            out=e2i[:, :, kcm, :], in0=ps2,
                            scalar1=EXPA, scalar2=EXPB, op0=OP.mult, op1=OP.add,
                        )
                    else:
                        nc.scalar.activation(
                            e2t[:, :, kcm, :], ps2, AF.Exp, bias=ln2t, scale=SCALE,
                        )
                e2s[kcp] = e2t
            if riders:
                riders.pop(0)()
            if kcp >= 1:
                p_ = kcp - 1
                st, sp = (p_ == 0), (p_ == KCP - 1)
                for j in range(2):
                    nc.tensor.matmul(
                        pcc[j][0 : DH + 1, :],
                        va[:, p_, :, 2 * hp + j, 0 : DH + 1],
                        e2s[p_][:, j, :, :],
                        start=st, stop=sp,
                        perf_mode=DR,
                        skip_group_check=True,
                    )
        for j in range(2):
            se = small.tile([1, NT], F32, tag="sm", name=f"se_{tq}_{hp}_{j}")
            nc.vector.tensor_copy(out=se, in_=pcc[j][DH : DH + 1, :])
            rc = small.tile([1, NT], F32, tag="sm", name=f"rc_{tq}_{hp}_{j}")
            nc.vector.reciprocal_approx_fast(out=rc, in_=se)
            db = chunk.tile([DH, NT], F32, tag="db", name=f"db_{tq}_{hp}_{j}")
            nc.gpsimd.partition_broadcast(db, rc)
            nc.vector.tensor_tensor(
                out=ctx[j * DH : (j + 1) * DH, hp, :],
                in0=pcc[j][0:DH, :],
                in1=db,
                op=OP.mult,
            )

    def oproj(tq, resid, co):
        ts_ = slice(tq * NT, (tq + 1) * NT)
        ctx = ctxs[tq]
        ps = psG.tile([P, NT], F32, tag="pg", name=f"ops_{tq}_{co}")
        for kp in range(2):
            nc.tensor.matmul(
                ps,
                wo[:, kp, :, co * P : (co + 1) * P],
                ctx[:, 2 * kp : 2 * kp + 2, :],
                start=(kp == 0),
                stop=(kp == 1),
                perf_mode=DR,
                skip_group_check=True,
            )
        nc.vector.scalar_tensor_tensor(
            out=resid[:, co, :], in0=ps, scalar=bo[:, co : co + 1],
            in1=xqr[:, co, ts_], op0=OP.add, op1=OP.add,
        )

    # ---------------- layernorm (stats in bf16, rsqrt via Newton) --------
    def layernorm(resid, g, be, out_write, tag, out_write_co=None):
        """Normalizes resid IN PLACE (except the final +be which out_write
        directs). resid: [P, C, NT] f32 tile."""
        rb = sbpool.tile([P, C, NT], BF16, tag="rb", name=f"rb_{tag}")
        nc.vector.tensor_copy(out=rb, in_=resid)
        s4 = sbpool.tile([P, C, NT], BF16, tag="rb", name=f"sq_{tag}")
        nc.vector.tensor_mul(s4, rb, rb)
        lnp = psX.tile([P, NT], F32, tag="px", name=f"lnp_{tag}")
        for co in range(C):
            nc.tensor.matmul(lnp[0:1, :], onesb, rb[:, co, :], start=(co == 0),
                             stop=(co == C - 1), skip_group_check=True)
        for co in range(C):
            nc.tensor.matmul(lnp[64:65, :], onesb, s4[:, co, :], start=(co == 0),
                             stop=(co == C - 1), tile_position=(0, 64),
                             skip_group_check=True)
        mean = small.tile([1, NT], F32, tag="sm", name=f"mean_{tag}")
        nc.vector.tensor_scalar_mul(mean, lnp[0:1, :], 1.0 / D)
        msq = small.tile([1, NT], F32, tag="sm", name=f"msq_{tag}")
        nc.vector.tensor_scalar_mul(msq, lnp[64:65, :], 1.0 / D)
        m2 = small.tile([1, NT], F32, tag="sm", name=f"m2_{tag}")
        nc.vector.tensor_mul(m2, mean, mean)
        var = small.tile([1, NT], F32, tag="sm", name=f"var_{tag}")
        nc.vector.tensor_tensor(out=var, in0=msq, in1=m2, op=OP.subtract)
        # rstd = exp(-0.5 * ln(var + eps)) -- stays in the Exp/Ln ACT table set
        lnv = small.tile([1, NT], F32, tag="sm", name=f"lnv_{tag}")
        nc.scalar.activation(lnv, var, AF.Ln, bias=epst)
        rstd = small.tile([1, NT], F32, tag="sm", name=f"rstd_{tag}")
        nc.scalar.activation(rstd, lnv, AF.Exp, scale=-0.5)
        meanb = chunk.tile([P, NT], F32, tag="bc", name=f"meanb_{tag}")
        nc.gpsimd.partition_broadcast(meanb, mean)
        rstdb = chunk.tile([P, NT], F32, tag="bc", name=f"rstdb_{tag}")
        nc.gpsimd.partition_broadcast(rstdb, rstd)
        if out_write_co is not None:
            for co in range(C):
                nc.vector.tensor_tensor(
                    out=resid[:, co, :], in0=resid[:, co, :], in1=meanb,
                    op=OP.subtract,
                )
                nc.vector.scalar_tensor_tensor(
                    out=resid[:, co, :], in0=resid[:, co, :],
                    scalar=g[:, co : co + 1], in1=rstdb, op0=OP.mult, op1=OP.mult,
                )
                out_write_co(co, resid[:, co, :], be[:, co : co + 1])
            return
        nc.vector.tensor_tensor(
            out=resid, in0=resid,
            in1=meanb[:, None, :].to_broadcast((P, C, NT)), op=OP.subtract,
        )
        nc.vector.tensor_tensor(
            out=resid, in0=resid,
            in1=rstdb[:, None, :].to_broadcast((P, C, NT)), op=OP.mult,
        )
        nc.vector.tensor_tensor(
            out=resid, in0=resid,
            in1=g[:, :, None].to_broadcast((P, C, NT)), op=OP.mult,
        )
        out_write(resid, be)

    # ---------------- FFN ----------------
    def ffn1(tq, ln1b, hb):
        for fp in range(CF // 2):
            ps = psS.tile([P, 2, NT], F32, tag="ps2", name=f"fps_{tq}_{fp}")
            for f2 in range(2):
                fo = 2 * fp + f2
                for kp in range(2):
                    nc.tensor.matmul(
                        ps[:, f2, :],
                        w1[:, kp, :, fo * P : (fo + 1) * P],
                        ln1b[:, 2 * kp : 2 * kp + 2, :],
                        start=(kp == 0),
                        stop=(kp == 1),
                        perf_mode=DR,
                    )
            for f2 in range(2):
                nc.scalar.activation(
                    hb[:, 2 * fp + f2, :], ps[:, f2, :], AF.Gelu,
                    bias=b1[:, 2 * fp + f2 : 2 * fp + f2 + 1],
                )

    def ffn2(tq, ln1f, hb):
        resid2 = stage1.tile([P, C, NT], F32, tag="resid2", name=f"resid2_{tq}")
        for co in range(C):
            ps = psX.tile([P, NT], F32, tag="px", name=f"gps_{tq}_{co}")
            for kp in range(CF // 2):
                nc.tensor.matmul(
                    ps,
                    w2[:, kp, :, co * P : (co + 1) * P],
                    hb[:, 2 * kp : 2 * kp + 2, :],
                    start=(kp == 0),
                    stop=(kp == CF // 2 - 1),
                    perf_mode=DR,
                    skip_group_check=True,
                )
            nc.vector.scalar_tensor_tensor(
                out=resid2[:, co, :], in0=ps, scalar=b2[:, co : co + 1],
                in1=ln1f[:, co, :], op0=OP.add, op1=OP.add,
            )
        return resid2

    def ln2(tq, resid2):
        ts_ = slice(tq * NT, (tq + 1) * NT)

        def write_out_co(co, v, bec, ts_=ts_):
            nc.vector.tensor_scalar(
                out=v, in0=v, scalar1=bec, scalar2=None, op0=OP.add
            )
            nc.sync.dma_start(out_d[:, co, ts_], v)

        layernorm(resid2, g2, be2, None, f"l2_{tq}", out_write_co=write_out_co)

    # ================= schedule =================
    for tt in range(4):
        for co in range(C):
            kproj(tt, co)
    qtp[0] = [ptq.tile([64, 2, NT], FP8, tag=f"qtp{i}", name=f"qtp_0_{i}")
              for i in range(C)]
    for co in range(C):
        qproj_co(0, co)
    qtp[1] = [ptq.tile([64, 2, NT], FP8, tag=f"qtp{i}", name=f"qtp_1_{i}")
              for i in range(C)]
    for co in range(C):
        qproj_co(1, co)
    for tm in range(KC):
        vproj(tm)

    # ---- attention tq0; Qproj(tq1) rides ----
    ctxs[0] = ptq.tile([P, C, NT], FP8, tag="ctx", name="ctx_0")
    nop = lambda: None
    for hp in range(H // 2):
        attn(0, hp)

    # ---- attention tq1; Oproj(0) + LN1(0) ride ----
    ctxs[1] = ptq.tile([P, C, NT], FP8, tag="ctx", name="ctx_1")
    resid0 = stage.tile([P, C, NT], F32, tag="resid", name="resid_0")
    ln1f0 = stage.tile([P, C, NT], F32, tag="ln1f", name="ln1f_0")
    ln1b0 = ptq.tile([P, C, NT], FP8, tag="ln1b", name="ln1b_0")

    def write_ln1_0(tt, be):
        nc.vector.tensor_tensor(
            out=ln1f0, in0=tt,
            in1=be[:, :, None].to_broadcast((P, C, NT)), op=OP.add,
        )
        nc.vector.tensor_copy(out=ln1b0, in_=ln1f0)

    for hp in range(H // 2):
        riders = [nop] * KCP
        if hp == 0:
            riders[2] = lambda: oproj(0, resid0, 0)
            riders[5] = lambda: oproj(0, resid0, 1)
        elif hp == 1:
            riders[2] = lambda: oproj(0, resid0, 2)
            riders[5] = lambda: oproj(0, resid0, 3)
        elif hp == 2:
            riders[3] = lambda: layernorm(resid0, g1, be1, write_ln1_0, "l1_0")
        attn(1, hp, riders)

    # ---- tails: FFN + LN chains, interleaved so each serial LN chain
    # hides under the next block's matmuls ----
    resid1 = stage.tile([P, C, NT], F32, tag="resid", name="resid_1")
    for co in range(C):
        oproj(1, resid1, co)

    ln1f1 = stage.tile([P, C, NT], F32, tag="ln1f", name="ln1f_1")
    ln1b1 = ptq.tile([P, C, NT], FP8, tag="ln1b", name="ln1b_1")

    def write_ln1_1(tt, be):
        nc.vector.tensor_tensor(
            out=ln1f1, in0=tt,
            in1=be[:, :, None].to_broadcast((P, C, NT)), op=OP.add,
        )
        nc.vector.tensor_copy(out=ln1b1, in_=ln1f1)

    layernorm(resid1, g1, be1, write_ln1_1, "l1_1")
    hb0 = hpool.tile([P, CF, NT], FP8, tag="h", name="h_0")
    ffn1(0, ln1b0, hb0)
    r2_0 = ffn2(0, ln1f0, hb0)
    ln2(0, r2_0)
    hb1 = hpool.tile([P, CF, NT], FP8, tag="h", name="h_1")
    ffn1(1, ln1b1, hb1)
    r2_1 = ffn2(1, ln1f1, hb1)
    ln2(1, r2_1)


@functools.lru_cache(maxsize=1)
def build():
    from contextlib import ExitStack

    nc = bacc.Bacc("TRN2", target_bir_lowering=False, debug=False, num_devices=NCORES)
    t = {}

    def din(name, shape, dt):
        t[name] = nc.dram_tensor(name, list(shape), dt, kind="ExternalInput").ap()

    din("xq8", (P, 2, 2, TQ), FP8)
    din("xqr", (P, C, TQ), BF16)
    din("xk8", (P, 2, 2, TK), FP8)
    din("xv8", (P, 2, 2, TK), FP8)
    for w in ("wq8", "wk8", "wv8", "wo8"):
        din(w, (P, 2, 2, D), FP8)
    din("w18", (P, 2, 2, FF), FP8)
    din("w28", (P, CF // 2, 2, D), FP8)
    din("ball", (P, 56), F32)
    t["out"] = nc.dram_tensor("out", [D, TQ], F32, kind="ExternalOutput").ap()

    with tile.TileContext(nc) as tc:
        with ExitStack() as es:
            _emit(nc, t, es, tc)
    nc.compile()
    return nc


def make_in_maps(query, key, value, Wq, bq, Wk, bk, Wv, bv, Wo, bo,
                 g1, be1, g2, be2, W1, b1, W2, b2):
    bf = ml_dtypes.bfloat16
    f8 = ml_dtypes.float8_e4m3

    def pmaj(w, dt=bf):
        # [K, N] -> partition-major [128, K//128, N]
        w = np.asarray(w)
        k, n = w.shape
        return np.ascontiguousarray(
            w.reshape(k // P, P, n).transpose(1, 0, 2).astype(dt)
        )

    def pmaj_dr(w):
        # [K, N] -> DoubleRow layout [128, K//256, 2, N] fp8
        w = np.asarray(w, np.float32)
        k, n = w.shape
        return np.ascontiguousarray(
            np.clip(w, -240, 240).reshape(k // 256, 2, P, n)
            .transpose(2, 0, 1, 3).astype(f8)
        )

    # column permutation packing each 128-feature chunk (2 heads) for the
    # DoubleRow scores layout: [h0 f0:32 | h1 f0:32 | h0 f32:64 | h1 f32:64]
    qk_perm = np.concatenate([
        c * 128 + np.concatenate([np.arange(s * 32, s * 32 + 32) + 64 * j
                                  for s in range(2) for j in range(2)])
        for c in range(4)
    ])
    bo2 = np.asarray(bo, np.float32) + np.asarray(bv, np.float32) @ np.asarray(Wo, np.float32)
    cols = [np.asarray(v, np.float32).reshape(-1, P).T
            for v in (bq, bk, bo2, b2, g1, be1, g2, be2, b1)]
    # bq in packed order: [64, 8] columns (2*co + s), zero-padded to 128 rows
    bq_pk = np.asarray(bq, np.float32)[qk_perm].reshape(4, 2, 64)
    bqp = np.zeros((P, 8), np.float32)
    bqp[0:64] = bq_pk.transpose(2, 0, 1).reshape(64, 8)
    cols.append(bqp)
    ball = np.ascontiguousarray(np.concatenate(cols, axis=1))  # [128, 56]
    shared = {
        "ball": ball,
        "wq8": pmaj_dr(np.asarray(Wq, np.float32)[:, qk_perm]),
        "wk8": pmaj_dr(np.asarray(Wk, np.float32)[:, qk_perm]),
        "wv8": pmaj_dr(Wv),
        "wo8": pmaj_dr(Wo),
        "w18": pmaj_dr(W1), "w28": pmaj_dr(W2),
    }
    in_maps = []
    for core in range(NCORES):
        b, half = divmod(core, 2)
        qsl = slice(half * TQ, (half + 1) * TQ)
        xq_t = np.asarray(query[b, qsl], np.float32).T  # [D, TQ]
        in_maps.append({
            "xq8": pmaj_dr(xq_t), "xqr": pmaj(xq_t),
            "xk8": pmaj_dr(np.asarray(key[b], np.float32).T),
            "xv8": pmaj_dr(np.asarray(value[b], np.float32).T), **shared,
        })
    return in_maps


def kernel(**inputs):
    nc = build()
    in_maps = make_in_maps(**inputs)
    res = run_bass_kernel_spmd(nc, in_maps, list(range(NCORES)))
    out = np.empty((B, S, D), np.float32)
    for core in range(NCORES):
        b, half = divmod(core, 2)
        out[b, half * TQ : (half + 1) * TQ] = res.results[core]["out"].T
    return out


if __name__ == "__main__":
    import reference

    inputs = {k: np.asarray(v) for k, v in reference.setup_inputs().items()}
    got = kernel(**inputs)
    exp = np.asarray(reference.reference(**inputs))
    err = np.abs(got - exp).max() / np.abs(exp).max()
    print("rel err:", err)



# revision 33
# speedup vs baseline: 1.2157x; 1.1211x over previous
"""Trainium2 Bass kernel for a cross-modal transformer block (attention + FFN).

Contract: kernel(**inputs) takes the FULL unsharded inputs (numpy, fp32) and
returns the FULL output [4, 2048, 512] fp32.

Sharding: 8 cores = data-parallel over batch (4) x query-sequence halves (2).
Each core computes K/V projections for its batch's full 2048-token sequence
(cheap duplication) so attention needs no collectives.

v2: fp8 (e4m3) everywhere on the attention side with DoubleRow matmuls for
the K>=256 contractions (QKVO projections, ctx, sumexp); part of the softmax
exp runs on the vector engine via a Schraudolph-style bit trick whose integer
output bits ARE the fp8 exp values; LayerNorm statistics matmuls run in bf16
and rsqrt is computed with a Newton iteration on the vector engine so the
scalar engine only ever loads the Exp and Gelu table sets. The FFN stays in
bf16 for accuracy headroom.
"""

import functools
import sys

import numpy as np

sys.path.insert(0, "/opt/trn_rl_repo")

import ml_dtypes  # noqa: E402

import concourse.bass as bass  # noqa: E402
import concourse.tile as tile  # noqa: E402
from concourse import bacc, mybir  # noqa: E402
from concourse.bass_utils import run_bass_kernel_spmd  # noqa: E402

_orig_tables = bacc.get_activation_tables


def _patched_tables(arch):
    tabs = dict(_orig_tables(arch))
    for name in ("exp_and_others", "exp_and_friends", "natural_log"):
        if name in tabs and "natural_log_exp_and_others" in tabs:
            tabs[name] = set()
    return tabs


bacc.get_activation_tables = _patched_tables

BF16 = mybir.dt.bfloat16
F32 = mybir.dt.float32
FP8 = mybir.dt.float8e4
I32 = mybir.dt.int32
I8 = mybir.dt.int8
AF = mybir.ActivationFunctionType
OP = mybir.AluOpType
DR = mybir.MatmulPerfMode.DoubleRow

B, S, D = 4, 2048, 512
H, DH = 8, 64
FF = 2048
P = 128
C = D // P  # 4 feature chunks
CF = FF // P  # 16 ffn chunks
TQ = S // 2  # 1024 query tokens per core
TK = S  # full key sequence per core
KC = TK // P  # 16 key chunks
KCP = KC // 2  # 8 key chunk pairs (DoubleRow)
NT = 512  # token tile (matmul free dim)
SCALE = 1.0 / np.sqrt(DH)  # 0.125
LN_EPS = 1e-5
NCORES = 8
LN2F = float(np.log(2.0))

# Schraudolph fast-exp constants for fp8e4m3 output bits:
#   bits = round(EXPA * raw_score + EXPB)  ->  ~ 2*exp(raw_score/8)
EXPA = float(8 * np.log2(np.e) * SCALE)
EXPB = 63.62
# key chunks whose exp runs on the vector / gpsimd engine instead of ACT
DVE_KC = (1, 4, 7, 10, 12, 14)
GPS_KC = ()

RSQRT_MAGIC = 0x5F3759DF


def _emit(nc, t, es, tc):
    """Emit the per-core program. t: dict name -> DRAM AP."""
    # ---------------- pools ----------------
    wp = es.enter_context(tc.tile_pool(name="w", bufs=1))
    ap_ = es.enter_context(tc.tile_pool(name="acts", bufs=1))
    ptq = es.enter_context(tc.tile_pool(name="ptq", bufs=2))
    # single PSUM pool: 4 x [P, 2, NT] tiles = all 8 banks.  The scores
    # pipeline cycles through whatever pcc/riders leave free (depth 2-3).
    psS = es.enter_context(tc.tile_pool(name="psS", bufs=4, space="PSUM"))
    epool = es.enter_context(tc.tile_pool(name="e", bufs=3))
    stage = es.enter_context(tc.tile_pool(name="stage", bufs=2))
    stage1 = es.enter_context(tc.tile_pool(name="stage1", bufs=2))
    sbpool = es.enter_context(tc.tile_pool(name="sb", bufs=2))
    chunk = es.enter_context(tc.tile_pool(name="chunk", bufs=4))
    small = es.enter_context(tc.tile_pool(name="small", bufs=8))
    hpool = es.enter_context(tc.tile_pool(name="h", bufs=2))

    # ---------------- DMA: params + inputs ----------------
    ball = wp.tile([P, 48], F32, name="ball")
    nc.sync.dma_start(ball, t["ball"])
    bq, bk, bo, b2 = (ball[:, 4 * i : 4 * (i + 1)] for i in range(4))
    g1, be1, g2, be2 = (ball[:, 16 + 4 * i : 20 + 4 * i] for i in range(4))
    b1 = ball[:, 32:48]

    def ld(pool, name, shape, dt, split=True):
        w = pool.tile([P] + list(shape), dt, name=name + "_sb")
        if split:
            nc.sync.dma_start(w[0:64], t[name][0:64])
            nc.sync.dma_start(w[64:P], t[name][64:P])
        else:
            nc.sync.dma_start(w, t[name])
        return w

    wk = ld(wp, "wk8", [2, 2, D], FP8, split=False)
    xk = wp.tile([P, 2, 2, TK], FP8, name="xk8_sb")
    for _tt in range(4):
        _ts = slice(_tt * NT, (_tt + 1) * NT)
        nc.sync.dma_start(xk[:, :, :, _ts], t["xk8"][:, :, :, _ts])
    wq = ld(wp, "wq8", [2, 2, D], FP8, split=False)
    xq8 = ld(wp, "xq8", [2, 2, TQ], FP8, split=False)
    wv = ld(wp, "wv8", [2, 2, D], FP8, split=False)
    xv = ld(wp, "xv8", [2, 2, TK], FP8)
    wo = ld(wp, "wo8", [2, 2, D], FP8, split=False)
    xqr = ld(wp, "xqr", [C, TQ], BF16)
    w1 = ld(wp, "w18", [2, 2, FF], FP8)
    w2 = ld(wp, "w28", [CF // 2, 2, D], FP8)

    onesb = wp.tile([P, 1], BF16)
    nc.vector.memset(onesb, 1.0)
    ln2t = wp.tile([P, 1], F32)
    nc.vector.memset(ln2t, LN2F)
    epst = wp.tile([1, 1], F32)
    nc.vector.memset(epst, LN_EPS)

    # persistent activations
    kts = [ap_.tile([P, TK], FP8, name=f"kt_{i}") for i in range(C)]
    va = ap_.tile([P, KCP, 2, H, 72], FP8, name="va")  # V token-major, padded
    nc.vector.memset(va[:, :, :, :, 64:65], 1.0)  # ones column -> sumexp row
    qts = [None, None]
    ctxs = [None, None]

    out_d = t["out"].rearrange("(c p) q -> p c q", p=P)

    # ---------------- projections ----------------
    def kproj(tt, co):
        ts_ = slice(tt * NT, (tt + 1) * NT)
        pst = psS.tile([P, 2, NT], F32, tag="ps2", name=f"kps_{tt}_{co}")
        ps = pst[:, 0, :]
        for kp in range(2):
            nc.tensor.matmul(
                ps,
                wk[:, kp, :, co * P : (co + 1) * P],
                xk[:, kp, :, ts_],
                start=(kp == 0),
                stop=(kp == 1),
                perf_mode=DR,
            )
        nc.scalar.activation(kts[co][:, ts_], ps, AF.Copy)

    def vproj(tm):
        msl = slice(tm * P, (tm + 1) * P)
        pst = psS.tile([P, 2, NT], F32, tag="ps2", name=f"vps_{tm}")
        ps = pst[:, 0, :]
        for kp in range(2):
            nc.tensor.matmul(
                ps,
                xv[:, kp, :, msl],
                wv[:, kp, :, :],
                start=(kp == 0),
                stop=(kp == 1),
                perf_mode=DR,
            )
        nc.scalar.activation(
            va[:, tm // 2, tm % 2, :, 0:DH],
            ps.rearrange("p (h d) -> p h d", h=H), AF.Copy,
        )

    def qproj_co(tq, co):
        qt = qts[tq]
        ts_ = slice(tq * NT, (tq + 1) * NT)
        pst = psS.tile([P, 2, NT], F32, tag="ps2", name=f"qps_{tq}_{co}")
        ps = pst[:, 0, :]
        for kp in range(2):
            nc.tensor.matmul(
                ps,
                wq[:, kp, :, co * P : (co + 1) * P],
                xq8[:, kp, :, ts_],
                start=(kp == 0),
                stop=(kp == 1),
                perf_mode=DR,
            )
        nc.vector.tensor_scalar(
            out=qt[:, co, :], in0=ps, scalar1=bq[:, co : co + 1],
            scalar2=None, op0=OP.add,
        )

    # ---------------- attention ----------------
    def attn(tq, hp, riders=None):
        qt = qts[tq]
        ctx = ctxs[tq]
        pcct = psS.tile([P, 2, NT], F32, tag="ps2", name=f"pcc_{tq}_{hp}")
        pcc = [pcct[:, j, :] for j in range(2)]
        e2s = [None] * KCP
        for kcp in range(KCP + 1):
            if kcp < KCP:
                e2t = epool.tile([P, 2, 2, NT], FP8, tag="e", name=f"e_{tq}_{hp}_{kcp}")
                for kcm in range(2):
                    kc = 2 * kcp + kcm
                    ksl = slice(kc * P, (kc + 1) * P)
                    ps2 = psS.tile([P, 2, NT], F32, tag="ps2", name=f"s_{tq}_{hp}_{kc}")
                    for j in range(2):
                        rows = slice(j * DH, (j + 1) * DH)
                        nc.tensor.matmul(
                            ps2[:, j, :], kts[hp][rows, ksl], qt[rows, hp, :],
                            start=True, stop=True,
                        )
                    if kc in DVE_KC:
                        e2i = e2t.bitcast(I8)
                        nc.vector.tensor_scalar(
                            out=e2i[:, :, kcm, :], in0=ps2,
                            scalar1=EXPA, scalar2=EXPB, op0=OP.mult, op1=OP.add,
                        )
                    elif kc in GPS_KC:
                        e2i = e2t.bitcast(I8)
                        nc.gpsimd.tensor_scalar(
                            out=e2i[:, :, kcm, :], in0=ps2,
                            scalar1=EXPA, scalar2=EXPB, op0=OP.mult, op1=OP.add,
                        )
                    else:
                        nc.scalar.activation(
                            e2t[:, :, kcm, :], ps2, AF.Exp, bias=ln2t, scale=SCALE,
                        )
                e2s[kcp] = e2t
            if riders:
                riders.pop(0)()
            if kcp >= 1:
                p_ = kcp - 1
                st, sp = (p_ == 0), (p_ == KCP - 1)
                for j in range(2):
                    nc.tensor.matmul(
                        pcc[j][0 : DH + 1, :],
                        va[:, p_, :, 2 * hp + j, 0 : DH + 1],
                        e2s[p_][:, j, :, :],
                        start=st, stop=sp,
                        perf_mode=DR,
                        skip_group_check=True,
                    )
        for j in range(2):
            se = small.tile([1, NT], F32, tag="sm", name=f"se_{tq}_{hp}_{j}")
            nc.vector.tensor_copy(out=se, in_=pcc[j][DH : DH + 1, :])
            rc = small.tile([1, NT], F32, tag="sm", name=f"rc_{tq}_{hp}_{j}")
            nc.vector.reciprocal_approx_fast(out=rc, in_=se)
            db = chunk.tile([DH, NT], F32, tag="db", name=f"db_{tq}_{hp}_{j}")
            nc.gpsimd.partition_broadcast(db, rc)
            nc.vector.tensor_tensor(
                out=ctx[j * DH : (j + 1) * DH, hp, :],
                in0=pcc[j][0:DH, :],
                in1=db,
                op=OP.mult,
            )

    def oproj(tq, resid, co):
        ts_ = slice(tq * NT, (tq + 1) * NT)
        ctx = ctxs[tq]
        pst = psS.tile([P, 2, NT], F32, tag="ps2", name=f"ops_{tq}_{co}")
        ps = pst[:, 0, :]
        for kp in range(2):
            nc.tensor.matmul(
                ps,
                wo[:, kp, :, co * P : (co + 1) * P],
                ctx[:, 2 * kp : 2 * kp + 2, :],
                start=(kp == 0),
                stop=(kp == 1),
                perf_mode=DR,
                skip_group_check=True,
            )
        nc.vector.scalar_tensor_tensor(
            out=resid[:, co, :], in0=ps, scalar=bo[:, co : co + 1],
            in1=xqr[:, co, ts_], op0=OP.add, op1=OP.add,
        )

    # ---------------- layernorm (stats in bf16, rsqrt via Newton) --------
    def layernorm(resid, g, be, out_write, tag, out_write_co=None):
        """Normalizes resid IN PLACE (except the final +be which out_write
        directs). resid: [P, C, NT] f32 tile."""
        rb = sbpool.tile([P, C, NT], BF16, tag="rb", name=f"rb_{tag}")
        nc.vector.tensor_copy(out=rb, in_=resid)
        s4 = sbpool.tile([P, C, NT], BF16, tag="rb", name=f"sq_{tag}")
        nc.vector.tensor_mul(s4, rb, rb)
        lnpt = psS.tile([P, 2, NT], F32, tag="ps2", name=f"lnp_{tag}")
        lnp = lnpt[:, 0, :]
        for co in range(C):
            nc.tensor.matmul(lnp[0:1, :], onesb, rb[:, co, :], start=(co == 0),
                             stop=(co == C - 1), skip_group_check=True)
        for co in range(C):
            nc.tensor.matmul(lnp[64:65, :], onesb, s4[:, co, :], start=(co == 0),
                             stop=(co == C - 1), tile_position=(0, 64),
                             skip_group_check=True)
        mean = small.tile([1, NT], F32, tag="sm", name=f"mean_{tag}")
        nc.vector.tensor_scalar_mul(mean, lnp[0:1, :], 1.0 / D)
        msq = small.tile([1, NT], F32, tag="sm", name=f"msq_{tag}")
        nc.vector.tensor_scalar_mul(msq, lnp[64:65, :], 1.0 / D)
        m2 = small.tile([1, NT], F32, tag="sm", name=f"m2_{tag}")
        nc.vector.tensor_mul(m2, mean, mean)
        var = small.tile([1, NT], F32, tag="sm", name=f"var_{tag}")
        nc.vector.tensor_tensor(out=var, in0=msq, in1=m2, op=OP.subtract)
        # rstd = exp(-0.5 * ln(var + eps)) -- stays in the Exp/Ln ACT table set
        lnv = small.tile([1, NT], F32, tag="sm", name=f"lnv_{tag}")
        nc.scalar.activation(lnv, var, AF.Ln, bias=epst)
        rstd = small.tile([1, NT], F32, tag="sm", name=f"rstd_{tag}")
        nc.scalar.activation(rstd, lnv, AF.Exp, scale=-0.5)
        meanb = chunk.tile([P, NT], F32, tag="bc", name=f"meanb_{tag}")
        nc.gpsimd.partition_broadcast(meanb, mean)
        rstdb = chunk.tile([P, NT], F32, tag="bc", name=f"rstdb_{tag}")
        nc.gpsimd.partition_broadcast(rstdb, rstd)
        if out_write_co is not None:
            for co in range(C):
                nc.vector.tensor_tensor(
                    out=resid[:, co, :], in0=resid[:, co, :], in1=meanb,
                    op=OP.subtract,
                )
                nc.vector.scalar_tensor_tensor(
                    out=resid[:, co, :], in0=resid[:, co, :],
                    scalar=g[:, co : co + 1], in1=rstdb, op0=OP.mult, op1=OP.mult,
                )
                out_write_co(co, resid[:, co, :], be[:, co : co + 1])
            return
        nc.vector.tensor_tensor(
            out=resid, in0=resid,
            in1=meanb[:, None, :].to_broadcast((P, C, NT)), op=OP.subtract,
        )
        nc.vector.tensor_tensor(
            out=resid, in0=resid,
            in1=rstdb[:, None, :].to_broadcast((P, C, NT)), op=OP.mult,
        )
        nc.vector.tensor_tensor(
            out=resid, in0=resid,
            in1=g[:, :, None].to_broadcast((P, C, NT)), op=OP.mult,
        )
        out_write(resid, be)

    # ---------------- FFN ----------------
    def ffn1(tq, ln1b, hb):
        for fp in range(CF // 2):
            ps = psS.tile([P, 2, NT], F32, tag="ps2", name=f"fps_{tq}_{fp}")
            for f2 in range(2):
                fo = 2 * fp + f2
                for kp in range(2):
                    nc.tensor.matmul(
                        ps[:, f2, :],
                        w1[:, kp, :, fo * P : (fo + 1) * P],
                        ln1b[:, 2 * kp : 2 * kp + 2, :],
                        start=(kp == 0),
                        stop=(kp == 1),
                        perf_mode=DR,
                    )
            for f2 in range(2):
                nc.scalar.activation(
                    hb[:, 2 * fp + f2, :], ps[:, f2, :], AF.Gelu,
                    bias=b1[:, 2 * fp + f2 : 2 * fp + f2 + 1],
                )

    def ffn2(tq, ln1f, hb):
        resid2 = stage1.tile([P, C, NT], F32, tag="resid2", name=f"resid2_{tq}")
        for co in range(C):
            pst = psS.tile([P, 2, NT], F32, tag="ps2", name=f"gps_{tq}_{co}")
            ps = pst[:, 0, :]
            for kp in range(CF // 2):
                nc.tensor.matmul(
                    ps,
                    w2[:, kp, :, co * P : (co + 1) * P],
                    hb[:, 2 * kp : 2 * kp + 2, :],
                    start=(kp == 0),
                    stop=(kp == CF // 2 - 1),
                    perf_mode=DR,
                    skip_group_check=True,
                )
            nc.vector.scalar_tensor_tensor(
                out=resid2[:, co, :], in0=ps, scalar=b2[:, co : co + 1],
                in1=ln1f[:, co, :], op0=OP.add, op1=OP.add,
            )
        return resid2

    def ln2(tq, resid2):
        ts_ = slice(tq * NT, (tq + 1) * NT)

        def write_out_co(co, v, bec, ts_=ts_):
            nc.vector.tensor_scalar(
                out=v, in0=v, scalar1=bec, scalar2=None, op0=OP.add
            )
            nc.sync.dma_start(out_d[:, co, ts_], v)

        layernorm(resid2, g2, be2, None, f"l2_{tq}", out_write_co=write_out_co)

    # ================= schedule =================
    for tt in range(4):
        for co in range(C):
            kproj(tt, co)
    qts[0] = ptq.tile([P, C, NT], FP8, tag="qt", name="qt_0")
    for co in range(C):
        qproj_co(0, co)
    qts[1] = ptq.tile([P, C, NT], FP8, tag="qt", name="qt_1")
    for co in range(C):
        qproj_co(1, co)
    # vproj tm 0..9 up front; 10..15 ride inside attn(0, hp=0) so the PE
    # stream stays dense through the attention entry (keeps HAM at 8/8)
    for tm in range(10):
        vproj(tm)

    # ---- attention tq0; rest of Vproj rides ----
    ctxs[0] = ptq.tile([P, C, NT], FP8, tag="ctx", name="ctx_0")
    nop = lambda: None
    for hp in range(H // 2):
        riders = [nop] * KCP
        if hp == 0:
            for k in range(6):
                riders[k] = (lambda tm=10 + k: vproj(tm))
        attn(0, hp, riders)

    # ---- attention tq1; Oproj(0) + LN1(0) ride ----
    ctxs[1] = ptq.tile([P, C, NT], FP8, tag="ctx", name="ctx_1")
    resid0 = stage.tile([P, C, NT], F32, tag="resid", name="resid_0")
    ln1f0 = stage.tile([P, C, NT], F32, tag="ln1f", name="ln1f_0")
    ln1b0 = ptq.tile([P, C, NT], FP8, tag="ln1b", name="ln1b_0")

    def write_ln1_0(tt, be):
        nc.vector.tensor_tensor(
            out=ln1f0, in0=tt,
            in1=be[:, :, None].to_broadcast((P, C, NT)), op=OP.add,
        )
        nc.vector.tensor_copy(out=ln1b0, in_=ln1f0)

    for hp in range(H // 2):
        riders = [nop] * KCP
        if hp == 0:
            riders[2] = lambda: oproj(0, resid0, 0)
            riders[5] = lambda: oproj(0, resid0, 1)
        elif hp == 1:
            riders[2] = lambda: oproj(0, resid0, 2)
            riders[5] = lambda: oproj(0, resid0, 3)
        elif hp == 2:
            riders[3] = lambda: layernorm(resid0, g1, be1, write_ln1_0, "l1_0")
        attn(1, hp, riders)

    # ---- tails: FFN + LN chains, interleaved so each serial LN chain
    # hides under the next block's matmuls ----
    resid1 = stage.tile([P, C, NT], F32, tag="resid", name="resid_1")
    for co in range(C):
        oproj(1, resid1, co)

    ln1f1 = stage.tile([P, C, NT], F32, tag="ln1f", name="ln1f_1")
    ln1b1 = ptq.tile([P, C, NT], FP8, tag="ln1b", name="ln1b_1")

    def write_ln1_1(tt, be):
        nc.vector.tensor_tensor(
            out=ln1f1, in0=tt,
            in1=be[:, :, None].to_broadcast((P, C, NT)), op=OP.add,
        )
        nc.vector.tensor_copy(out=ln1b1, in_=ln1f1)

    layernorm(resid1, g1, be1, write_ln1_1, "l1_1")
    hb0 = hpool.tile([P, CF, NT], FP8, tag="h", name="h_0")
    ffn1(0, ln1b0, hb0)
    hb1 = hpool.tile([P, CF, NT], FP8, tag="h", name="h_1")
    ffn1(1, ln1b1, hb1)
    r2_0 = ffn2(0, ln1f0, hb0)
    ln2(0, r2_0)
    r2_1 = ffn2(1, ln1f1, hb1)
    ln2(1, r2_1)


@functools.lru_cache(maxsize=1)
def build():
    from contextlib import ExitStack

    nc = bacc.Bacc("TRN2", target_bir_lowering=False, debug=False, num_devices=NCORES)
    t = {}

    def din(name, shape, dt):
        t[name] = nc.dram_tensor(name, list(shape), dt, kind="ExternalInput").ap()

    din("xq8", (P, 2, 2, TQ), FP8)
    din("xqr", (P, C, TQ), BF16)
    din("xk8", (P, 2, 2, TK), FP8)
    din("xv8", (P, 2, 2, TK), FP8)
    for w in ("wq8", "wk8", "wv8", "wo8"):
        din(w, (P, 2, 2, D), FP8)
    din("w18", (P, 2, 2, FF), FP8)
    din("w28", (P, CF // 2, 2, D), FP8)
    din("ball", (P, 48), F32)
    t["out"] = nc.dram_tensor("out", [D, TQ], F32, kind="ExternalOutput").ap()

    with tile.TileContext(nc) as tc:
        with ExitStack() as es:
            _emit(nc, t, es, tc)
    nc.compile()
    return nc


def make_in_maps(query, key, value, Wq, bq, Wk, bk, Wv, bv, Wo, bo,
                 g1, be1, g2, be2, W1, b1, W2, b2):
    bf = ml_dtypes.bfloat16
    f8 = ml_dtypes.float8_e4m3

    def pmaj(w, dt=bf):
        # [K, N] -> partition-major [128, K//128, N]
        w = np.asarray(w)
        k, n = w.shape
        return np.ascontiguousarray(
            w.reshape(k // P, P, n).transpose(1, 0, 2).astype(dt)
        )

    def pmaj_dr(w):
        # [K, N] -> DoubleRow layout [128, K//256, 2, N] fp8
        w = np.asarray(w, np.float32)
        k, n = w.shape
        return np.ascontiguousarray(
            np.clip(w, -240, 240).reshape(k // 256, 2, P, n)
            .transpose(2, 0, 1, 3).astype(f8)
        )

    bo2 = np.asarray(bo, np.float32) + np.asarray(bv, np.float32) @ np.asarray(Wo, np.float32)
    cols = [np.asarray(v, np.float32).reshape(-1, P).T
            for v in (bq, bk, bo2, b2, g1, be1, g2, be2, b1)]
    ball = np.ascontiguousarray(np.concatenate(cols, axis=1))  # [128, 48]
    shared = {
        "ball": ball,
        "wq8": pmaj_dr(Wq), "wk8": pmaj_dr(Wk), "wv8": pmaj_dr(Wv),
        "wo8": pmaj_dr(Wo),
        "w18": pmaj_dr(W1), "w28": pmaj_dr(W2),
    }
    in_maps = []
    for core in range(NCORES):
        b, half = divmod(core, 2)
        qsl = slice(half * TQ, (half + 1) * TQ)
        xq_t = np.asarray(query[b, qsl], np.float32).T  # [D, TQ]
        in_maps.append({
            "xq8": pmaj_dr(xq_t), "xqr": pmaj(xq_t),
            "xk8": pmaj_dr(np.asarray(key[b], np.float32).T),
            "xv8": pmaj_dr(np.asarray(value[b], np.float32).T), **shared,
        })
    return in_maps


def kernel(**inputs):
    nc = build()
    in_maps = make_in_maps(**inputs)
    res = run_bass_kernel_spmd(nc, in_maps, list(range(NCORES)))
    out = np.empty((B, S, D), np.float32)
    for core in range(NCORES):
        b, half = divmod(core, 2)
        out[b, half * TQ : (half + 1) * TQ] = res.results[core]["out"].T
    return out


if __name__ == "__main__":
    import reference

    inputs = {k: np.asarray(v) for k, v in reference.setup_inputs().items()}
    got = kernel(**inputs)
    exp = np.asarray(reference.reference(**inputs))
    err = np.abs(got - exp).max() / np.abs(exp).max()
    print("rel err:", err)

